# revision 43
# baseline (speedup 1.0000x reference)
"""RBF Gram-matrix kernel for Trainium2 (8 NeuronCores, SPMD).

Computes out[n, m] = exp(-gamma * ||x_n - y_m||^2) for x: [8192, 512],
y: [8192, 512] via the GEMM identity ||x-y||^2 = x2 + y2 - 2*x.y.

Sharding: 4x2 grid over the 8 cores — x rows split in 4 shards of 2048,
y rows split in 2 shards of 4096. Each core computes a [2048, 4096] tile
of the full [8192, 8192] output.

Device kernel per core, using the multiplicative split
  exp(-g||x-y||^2) = exp(2g x.y - g x2[n] - S) * e^{S - g y2[m]}:
  psum[n, m]  = sum_d (2g*x)^T[d, n] * y^T[d, m]        (TensorE, fp8e4
                DoubleRow: 256-deep contraction per matmul, f32 acc)
  o1          = exp(psum + (-g*x2[n] - S))               (ScalarE LUT,
                reads PSUM, per-partition bias, bf16 out)
  o           = o1 * e^{S - g*y2}[m]                     (VectorE TT mult,
                all-bf16 SBUF -> 2x perf mode)
The shift S keeps the exp argument comfortably below bf16 overflow so the
product can never be Inf*0.  Output travels as bf16 (halves the 32 MB/core
drain DMA) and is upcast to f32 on the host.

Schedule: m-halves outer, n-chunks inner, one [128, 2048] PSUM tile
(4 banks, double-buffered) per iteration; the 2048-wide ACT drain is the
saturated engine (~99% busy steady-state); TensorE runs DoubleRow matmuls
at ~216 ns each with LDWEIGHTS fully hidden; output leaves as one
[128, 2048] bf16 DMA per iteration (4KB DRAM rows -> full-rate packets).
"""
import os
import time
from contextlib import ExitStack

import numpy as np
import ml_dtypes

import concourse.mybir as mybir
import concourse.tile as tile
from concourse import bacc
from concourse.bass_utils import run_bass_kernel_spmd

N, M, D = 8192, 8192, 512
XS, YS = 4, 2              # shard grid: 4 x-shards x 2 y-shards = 8 cores
NL, ML = N // XS, M // YS  # per-core output tile: [2048, 4096]
P = 128
DCH = D // P               # 4 contraction subtiles of 128
KB = DCH // 2              # 2 DoubleRow chunks, 256-deep each
NCH = NL // P              # 16 row chunks of 128
FREE = 512                 # matmul free dim = one PSUM bank of f32
MT = ML // FREE            # 8 column tiles
HB = 2                     # m-halves: 2 psum tiles of 4 banks per n-chunk
MH = MT // HB              # 4 m-tiles per half
HW = MH * FREE             # 2048 cols per half
SHIFT = 50.0               # exponent rebalance between the two factors

_CACHE = {}
last_results = None        # BassKernelResults of the most recent run (for test.py)


def _build_nc():
    nc = bacc.Bacc("TRN2", target_bir_lowering=False, debug=False, num_devices=8)
    xt_d = nc.dram_tensor("xt", [D, NL], mybir.dt.float8e4, kind="ExternalInput").ap()
    yt_d = nc.dram_tensor("yt", [D, ML], mybir.dt.float8e4, kind="ExternalInput").ap()
    ey2_d = nc.dram_tensor("ey2", [P, ML], mybir.dt.bfloat16, kind="ExternalInput").ap()
    bias_d = nc.dram_tensor("biast", [P, NCH], mybir.dt.float32, kind="ExternalInput").ap()
    out_d = nc.dram_tensor("out", [NL, ML], mybir.dt.bfloat16, kind="ExternalOutput").ap()

    with tile.TileContext(nc) as tc, ExitStack() as ctx:
        const = ctx.enter_context(tc.tile_pool(name="const", bufs=1))
        psum = ctx.enter_context(tc.tile_pool(name="psum", bufs=2, space="PSUM"))
        o1p = ctx.enter_context(tc.tile_pool(name="oexp", bufs=3))
        op = ctx.enter_context(tc.tile_pool(name="omul", bufs=3))

        xt_sb = const.tile([P, DCH, NL], mybir.dt.float8e4, tag="xt")
        yt_sb = const.tile([P, DCH, ML], mybir.dt.float8e4, tag="yt")
        ey2_sb = const.tile([P, ML], mybir.dt.bfloat16, tag="ey2")
        bias_sb = const.tile([P, NCH], mybir.dt.float32, tag="bias")

        xt_r = xt_d.rearrange("(c p) n -> p c n", p=P)
        yt_r = yt_d.rearrange("(c p) n -> p c n", p=P)

        # Input DMAs in first-use order with a fine-grained head so the first
        # matmuls are gated on as few bytes as possible: iteration (mh=0,n=0)
        # needs xt d-chunks 0-1 cols 0:512 and yt d-chunks 0-1 cols 0:2048
        # (b=0), then d-chunks 2-3 (b=1); the second m-half's yt and ey2
        # stream in behind the first half's compute.
        nc.sync.dma_start(bias_sb[:], bias_d[:])
        nc.sync.dma_start(xt_sb[:, 0:2, :FREE], xt_r[:, 0:2, :FREE])
        nc.sync.dma_start(yt_sb[:, 0:2, :FREE], yt_r[:, 0:2, :FREE])
        nc.sync.dma_start(yt_sb[:, 0:2, FREE:HW], yt_r[:, 0:2, FREE:HW])
        nc.sync.dma_start(yt_sb[:, 2:4, :HW], yt_r[:, 2:4, :HW])
        nc.sync.dma_start(xt_sb[:, 2:4, :FREE], xt_r[:, 2:4, :FREE])
        nc.sync.dma_start(xt_sb[:, 0:2, FREE:], xt_r[:, 0:2, FREE:])
        nc.sync.dma_start(xt_sb[:, 2:4, FREE:], xt_r[:, 2:4, FREE:])
        nc.sync.dma_start(ey2_sb[:, :HW], ey2_d[:, :HW])
        nc.sync.dma_start(yt_sb[:, 0:2, HW:], yt_r[:, 0:2, HW:])
        nc.sync.dma_start(yt_sb[:, 2:4, HW:], yt_r[:, 2:4, HW:])
        nc.sync.dma_start(ey2_sb[:, HW:], ey2_d[:, HW:])

        # m-halves as the outer loop: steady state needs only half of yt
        # loaded; the other half streams in during the first half's compute.
        for mh in range(HB):
            for n in range(NCH):
                pt = psum.tile([P, HW], mybir.dt.float32, tag="pt",
                               name=f"pt_{mh}_{n}")
                o1 = o1p.tile([P, HW], mybir.dt.bfloat16, tag="o1",
                              name=f"o1_{mh}_{n}")
                for b in range(KB):
                    for mi in range(MH):
                        m = mh * MH + mi
                        nc.tensor.matmul(
                            pt[:, mi * FREE:(mi + 1) * FREE],
                            xt_sb[:, 2 * b:2 * b + 2, n * P:(n + 1) * P],
                            yt_sb[:, 2 * b:2 * b + 2, m * FREE:(m + 1) * FREE],
                            start=(b == 0),
                            stop=(b == KB - 1),
                            perf_mode=mybir.MatmulPerfMode.DoubleRow,
                        )
                o = op.tile([P, HW], mybir.dt.bfloat16, tag="o",
                            name=f"o_{mh}_{n}")
                if mh < HB - 1 or n < NCH - 1:
                    nc.scalar.activation(
                        o1[:], pt[:],
                        mybir.ActivationFunctionType.Exp,
                        bias=bias_sb[:, n:n + 1], scale=1.0,
                    )
                    nc.vector.tensor_mul(
                        o[:], o1[:], ey2_sb[:, mh * HW:(mh + 1) * HW]
                    )
                    nc.sync.dma_start(
                        out_d[n * P:(n + 1) * P, mh * HW:(mh + 1) * HW], o[:]
                    )
                else:
                    # very last tile: half-grained epilogue shortens the
                    # final serial ACT -> DVE -> DMA drain
                    QW = HW // 2
                    for q in range(2):
                        sl = slice(q * QW, (q + 1) * QW)
                        nc.scalar.activation(
                            o1[:, sl], pt[:, sl],
                            mybir.ActivationFunctionType.Exp,
                            bias=bias_sb[:, n:n + 1], scale=1.0,
                        )
                        nc.vector.tensor_mul(
                            o[:, sl], o1[:, sl],
                            ey2_sb[:, mh * HW + q * QW:mh * HW + (q + 1) * QW],
                        )
                        nc.sync.dma_start(
                            out_d[n * P:(n + 1) * P,
                                  mh * HW + q * QW:mh * HW + (q + 1) * QW],
                            o[:, sl],
                        )

    nc.compile()
    return nc


def kernel(x, y, gamma):
    global last_results
    x = np.asarray(x, dtype=np.float32).reshape(N, D)
    y = np.asarray(y, dtype=np.float32).reshape(M, D)
    g = float(np.asarray(gamma, dtype=np.float32).reshape(-1)[0])

    x2 = np.einsum("nd,nd->n", x, x, dtype=np.float32)
    y2 = np.einsum("md,md->m", y, y, dtype=np.float32)
    xt = np.ascontiguousarray((x * np.float32(2.0 * g)).T).astype(ml_dtypes.float8_e4m3)
    yt = np.ascontiguousarray(y.T).astype(ml_dtypes.float8_e4m3)
    bias = (-g * x2 - np.float32(SHIFT)).astype(np.float32)       # ACT bias
    ey2 = np.exp(SHIFT - g * y2.astype(np.float64))               # column factor
    ey2 = ey2.astype(ml_dtypes.bfloat16)

    in_maps = []
    for k in range(8):
        i, j = divmod(k, YS)
        in_maps.append({
            "xt": np.ascontiguousarray(xt[:, i * NL:(i + 1) * NL]),
            "yt": np.ascontiguousarray(yt[:, j * ML:(j + 1) * ML]),
            "ey2": np.ascontiguousarray(
                np.broadcast_to(ey2[j * ML:(j + 1) * ML], (P, ML))
            ),
            "biast": np.ascontiguousarray(
                bias[i * NL:(i + 1) * NL].reshape(NCH, P).T
            ),
        })

    if "nc" not in _CACHE:
        _CACHE["nc"] = _build_nc()
    nc = _CACHE["nc"]

    trace = os.environ.get("KERNEL_TRACE", "0") == "1"
    last_results = run_bass_kernel_spmd(nc, in_maps, list(range(8)), trace=trace)

    out = np.empty((N, M), dtype=np.float32)
    for k in range(8):
        i, j = divmod(k, YS)
        out[i * NL:(i + 1) * NL, j * ML:(j + 1) * ML] = (
            last_results.results[k]["out"].astype(np.float32)
        )
    return out


if __name__ == "__main__":
    t0 = time.time()
    rng = np.random.default_rng(0)
    x = rng.standard_normal((N, D), dtype=np.float32)
    y = rng.standard_normal((M, D), dtype=np.float32)
    gamma = np.ones((1,), dtype=np.float32)
    out = kernel(x, y, gamma)
    print(f"kernel() wall: {time.time()-t0:.1f}s; out[0,:4]={out[0, :4]}")


# revision 45
# speedup vs baseline: 1.1634x; 1.1634x over previous
"""RBF Gram-matrix kernel for Trainium2 (8 NeuronCores, SPMD).

Computes out[n, m] = exp(-gamma * ||x_n - y_m||^2) for x: [8192, 512],
y: [8192, 512] via the GEMM identity ||x-y||^2 = x2 + y2 - 2*x.y.

Sharding: 4x2 grid over the 8 cores — x rows split in 4 shards of 2048,
y rows split in 2 shards of 4096. Each core computes a [2048, 4096] tile
of the full [8192, 8192] output.

Device kernel per core, using the multiplicative split
  exp(-g||x-y||^2) = exp(2g x.y - g x2[n] - S) * e^{S - g y2[m]}:
  psum[n, m]  = sum_d (2g*x)^T[d, n] * y^T[d, m]        (TensorE, fp8e4
                DoubleRow: 256-deep contraction per matmul, f32 acc)
  o1          = exp(psum + (-g*x2[n] - S))               (ScalarE LUT,
                reads PSUM, per-partition bias, bf16 out)
  o           = o1 * e^{S - g*y2}[m]                     (VectorE TT mult,
                all-bf16 SBUF -> 2x perf mode)
The shift S keeps the exp argument comfortably below bf16 overflow so the
product can never be Inf*0.  Output travels as bf16 (halves the 32 MB/core
drain DMA) and is upcast to f32 on the host.

Schedule: m-halves outer, n-chunks inner, one [128, 2048] PSUM tile
(4 banks, double-buffered) per iteration; the 2048-wide ACT drain is the
saturated engine (~99% busy steady-state); TensorE runs DoubleRow matmuls
at ~216 ns each with LDWEIGHTS fully hidden; output leaves as one
[128, 2048] bf16 DMA per iteration (4KB DRAM rows -> full-rate packets).
"""
import os
import time
from contextlib import ExitStack

import numpy as np
import ml_dtypes

import concourse.mybir as mybir
import concourse.tile as tile
from concourse import bacc
from concourse.bass_utils import run_bass_kernel_spmd

N, M, D = 8192, 8192, 512
XS, YS = 4, 2              # shard grid: 4 x-shards x 2 y-shards = 8 cores
NL, ML = N // XS, M // YS  # per-core output tile: [2048, 4096]
P = 128
DCH = D // P               # 4 contraction subtiles of 128
KB = DCH // 2              # 2 DoubleRow chunks, 256-deep each
NCH = NL // P              # 16 row chunks of 128
FREE = 512                 # matmul free dim = one PSUM bank of f32
MT = ML // FREE            # 8 column tiles
HB = 2                     # m-halves: 2 psum tiles of 4 banks per n-chunk
MH = MT // HB              # 4 m-tiles per half
HW = MH * FREE             # 2048 cols per half
SHIFT = 50.0               # exponent rebalance between the two factors

_CACHE = {}
last_results = None        # BassKernelResults of the most recent run (for test.py)


def _build_nc():
    nc = bacc.Bacc("TRN2", target_bir_lowering=False, debug=False, num_devices=8)
    xt_d = nc.dram_tensor("xt", [D, NL], mybir.dt.float8e4, kind="ExternalInput").ap()
    yt_d = nc.dram_tensor("yt", [D, ML], mybir.dt.float8e4, kind="ExternalInput").ap()
    ey2_d = nc.dram_tensor("ey2", [P, ML], mybir.dt.bfloat16, kind="ExternalInput").ap()
    bias_d = nc.dram_tensor("biast", [P, NCH], mybir.dt.float32, kind="ExternalInput").ap()
    out_d = nc.dram_tensor("out", [NL, ML], mybir.dt.bfloat16, kind="ExternalOutput").ap()

    with tile.TileContext(nc) as tc, ExitStack() as ctx:
        const = ctx.enter_context(tc.tile_pool(name="const", bufs=1))
        psum = ctx.enter_context(tc.tile_pool(name="psum", bufs=2, space="PSUM"))
        o1p = ctx.enter_context(tc.tile_pool(name="oexp", bufs=3))
        op = ctx.enter_context(tc.tile_pool(name="omul", bufs=3))

        xt_sb = const.tile([P, DCH, NL], mybir.dt.float8e4, tag="xt")
        yt_sb = const.tile([P, DCH, ML], mybir.dt.float8e4, tag="yt")
        ey2_sb = const.tile([P, ML], mybir.dt.bfloat16, tag="ey2")
        bias_sb = const.tile([P, NCH], mybir.dt.float32, tag="bias")

        xt_r = xt_d.rearrange("(c p) n -> p c n", p=P)
        yt_r = yt_d.rearrange("(c p) n -> p c n", p=P)

        # Input DMAs in first-use order with a fine-grained head so the first
        # matmuls are gated on as few bytes as possible: iteration (mh=0,n=0)
        # needs xt d-chunks 0-1 cols 0:512 and yt d-chunks 0-1 cols 0:2048
        # (b=0), then d-chunks 2-3 (b=1); the second m-half's yt and ey2
        # stream in behind the first half's compute.
        nc.sync.dma_start(bias_sb[:], bias_d[:])
        nc.sync.dma_start(xt_sb[:, 0:2, :FREE], xt_r[:, 0:2, :FREE])
        nc.sync.dma_start(yt_sb[:, 0:2, :FREE], yt_r[:, 0:2, :FREE])
        nc.sync.dma_start(yt_sb[:, 0:2, FREE:HW], yt_r[:, 0:2, FREE:HW])
        nc.sync.dma_start(yt_sb[:, 2:4, :HW], yt_r[:, 2:4, :HW])
        nc.sync.dma_start(xt_sb[:, 2:4, :FREE], xt_r[:, 2:4, :FREE])
        nc.sync.dma_start(xt_sb[:, 0:2, FREE:], xt_r[:, 0:2, FREE:])
        nc.sync.dma_start(xt_sb[:, 2:4, FREE:], xt_r[:, 2:4, FREE:])
        nc.sync.dma_start(ey2_sb[:, :HW], ey2_d[:, :HW])
        nc.sync.dma_start(yt_sb[:, 0:2, HW:], yt_r[:, 0:2, HW:])
        nc.sync.dma_start(yt_sb[:, 2:4, HW:], yt_r[:, 2:4, HW:])
        nc.sync.dma_start(ey2_sb[:, HW:], ey2_d[:, HW:])

        # m-halves as the outer loop: steady state needs only half of yt
        # loaded; the other half streams in during the first half's compute.
        for mh in range(HB):
            for n in range(NCH):
                pt = psum.tile([P, HW], mybir.dt.float32, tag="pt",
                               name=f"pt_{mh}_{n}")
                o1 = o1p.tile([P, HW], mybir.dt.bfloat16, tag="o1",
                              name=f"o1_{mh}_{n}")
                for b in range(KB):
                    for mi in range(MH):
                        m = mh * MH + mi
                        nc.tensor.matmul(
                            pt[:, mi * FREE:(mi + 1) * FREE],
                            xt_sb[:, 2 * b:2 * b + 2, n * P:(n + 1) * P],
                            yt_sb[:, 2 * b:2 * b + 2, m * FREE:(m + 1) * FREE],
                            start=(b == 0),
                            stop=(b == KB - 1),
                            perf_mode=mybir.MatmulPerfMode.DoubleRow,
                        )
                o = op.tile([P, HW], mybir.dt.bfloat16, tag="o",
                            name=f"o_{mh}_{n}")
                if mh < HB - 1 or n < NCH - 1:
                    nc.scalar.activation(
                        o1[:], pt[:],
                        mybir.ActivationFunctionType.Exp,
                        bias=bias_sb[:, n:n + 1], scale=1.0,
                    )
                    nc.vector.tensor_mul(
                        o[:], o1[:], ey2_sb[:, mh * HW:(mh + 1) * HW]
                    )
                    nc.sync.dma_start(
                        out_d[n * P:(n + 1) * P, mh * HW:(mh + 1) * HW], o[:]
                    )
                else:
                    # very last tile: half-grained epilogue shortens the
                    # final serial ACT -> DVE -> DMA drain
                    QW = HW // 2
                    for q in range(2):
                        sl = slice(q * QW, (q + 1) * QW)
                        nc.scalar.activation(
                            o1[:, sl], pt[:, sl],
                            mybir.ActivationFunctionType.Exp,
                            bias=bias_sb[:, n:n + 1], scale=1.0,
                        )
                        nc.vector.tensor_mul(
                            o[:, sl], o1[:, sl],
                            ey2_sb[:, mh * HW + q * QW:mh * HW + (q + 1) * QW],
                        )
                        nc.sync.dma_start(
                            out_d[n * P:(n + 1) * P,
                                  mh * HW + q * QW:mh * HW + (q + 1) * QW],
                            o[:, sl],
                        )

    nc.compile()
    return nc


def kernel(x, y, gamma):
    global last_results
    x = np.asarray(x, dtype=np.float32).reshape(N, D)
    y = np.asarray(y, dtype=np.float32).reshape(M, D)
    g = float(np.asarray(gamma, dtype=np.float32).reshape(-1)[0])

    x2 = np.einsum("nd,nd->n", x, x, dtype=np.float32)
    y2 = np.einsum("md,md->m", y, y, dtype=np.float32)
    xt = np.ascontiguousarray((x * np.float32(2.0 * g)).T).astype(ml_dtypes.float8_e4m3)
    yt = np.ascontiguousarray(y.T).astype(ml_dtypes.float8_e4m3)
    bias = (-g * x2 - np.float32(SHIFT)).astype(np.float32)       # ACT bias
    ey2 = np.exp(SHIFT - g * y2.astype(np.float64))               # column factor
    ey2 = ey2.astype(ml_dtypes.bfloat16)

    in_maps = []
    for k in range(8):
        i, j = divmod(k, YS)
        in_maps.append({
            "xt": np.ascontiguousarray(xt[:, i * NL:(i + 1) * NL]),
            "yt": np.ascontiguousarray(yt[:, j * ML:(j + 1) * ML]),
            "ey2": np.ascontiguousarray(
                np.broadcast_to(ey2[j * ML:(j + 1) * ML], (P, ML))
            ),
            "biast": np.ascontiguousarray(
                bias[i * NL:(i + 1) * NL].reshape(NCH, P).T
            ),
        })

    if "nc" not in _CACHE:
        _CACHE["nc"] = _build_nc()
    nc = _CACHE["nc"]

    trace = os.environ.get("KERNEL_TRACE", "0") == "1"
    last_results = run_bass_kernel_spmd(nc, in_maps, list(range(8)), trace=trace)

    out = np.empty((N, M), dtype=np.float32)
    for k in range(8):
        i, j = divmod(k, YS)
        out[i * NL:(i + 1) * NL, j * ML:(j + 1) * ML] = (
            last_results.results[k]["out"].astype(np.float32)
        )
    return out


if __name__ == "__main__":
    t0 = time.time()
    rng = np.random.default_rng(0)
    x = rng.standard_normal((N, D), dtype=np.float32)
    y = rng.standard_normal((M, D), dtype=np.float32)
    gamma = np.ones((1,), dtype=np.float32)
    out = kernel(x, y, gamma)
    print(f"kernel() wall: {time.time()-t0:.1f}s; out[0,:4]={out[0, :4]}")


# revision 46
# speedup vs baseline: 1.1637x; 1.0002x over previous
"""RBF Gram-matrix kernel for Trainium2 (8 NeuronCores, SPMD).

Computes out[n, m] = exp(-gamma * ||x_n - y_m||^2) for x: [8192, 512],
y: [8192, 512] via the GEMM identity ||x-y||^2 = x2 + y2 - 2*x.y.

Sharding: 4x2 grid over the 8 cores — x rows split in 4 shards of 2048,
y rows split in 2 shards of 4096. Each core computes a [2048, 4096] tile
of the full [8192, 8192] output.

Device kernel per core, using the multiplicative split
  exp(-g||x-y||^2) = exp(2g x.y - g x2[n] - S) * e^{S - g y2[m]}:
  psum[n, m]  = sum_d (2g*x)^T[d, n] * y^T[d, m]        (TensorE, fp8e4
                DoubleRow: 256-deep contraction per matmul, f32 acc)
  o1          = exp(psum + (-g*x2[n] - S))               (ScalarE LUT,
                reads PSUM, per-partition bias, bf16 out)
  o           = o1 * e^{S - g*y2}[m]                     (VectorE TT mult,
                all-bf16 SBUF -> 2x perf mode)
The shift S keeps the exp argument comfortably below bf16 overflow so the
product can never be Inf*0.  Output travels as bf16 (halves the 32 MB/core
drain DMA) and is upcast to f32 on the host.

Schedule: m-halves outer, n-chunks inner, one [128, 2048] PSUM tile
(4 banks, double-buffered) per iteration; the 2048-wide ACT drain is the
saturated engine (~99% busy steady-state); TensorE runs DoubleRow matmuls
at ~216 ns each with LDWEIGHTS fully hidden; output leaves as one
[128, 2048] bf16 DMA per iteration (4KB DRAM rows -> full-rate packets).
"""
import os
import time
from contextlib import ExitStack

import numpy as np
import ml_dtypes

import concourse.mybir as mybir
import concourse.tile as tile
from concourse import bacc
from concourse.bass_utils import run_bass_kernel_spmd

N, M, D = 8192, 8192, 512
XS, YS = 4, 2              # shard grid: 4 x-shards x 2 y-shards = 8 cores
NL, ML = N // XS, M // YS  # per-core output tile: [2048, 4096]
P = 128
DCH = D // P               # 4 contraction subtiles of 128
KB = DCH // 2              # 2 DoubleRow chunks, 256-deep each
NCH = NL // P              # 16 row chunks of 128
FREE = 512                 # matmul free dim = one PSUM bank of f32
MT = ML // FREE            # 8 column tiles
HB = 2                     # m-halves: 2 psum tiles of 4 banks per n-chunk
MH = MT // HB              # 4 m-tiles per half
HW = MH * FREE             # 2048 cols per half
SHIFT = 50.0               # exponent rebalance between the two factors

_CACHE = {}
last_results = None        # BassKernelResults of the most recent run (for test.py)


def _build_nc():
    nc = bacc.Bacc("TRN2", target_bir_lowering=False, debug=False, num_devices=8)
    xt_d = nc.dram_tensor("xt", [D, NL], mybir.dt.float8e4, kind="ExternalInput").ap()
    yt_d = nc.dram_tensor("yt", [D, ML], mybir.dt.float8e4, kind="ExternalInput").ap()
    ey2_d = nc.dram_tensor("ey2", [P, ML], mybir.dt.bfloat16, kind="ExternalInput").ap()
    bias_d = nc.dram_tensor("biast", [P, NCH], mybir.dt.float32, kind="ExternalInput").ap()
    out_d = nc.dram_tensor("out", [NL, ML], mybir.dt.bfloat16, kind="ExternalOutput").ap()

    with tile.TileContext(nc) as tc, ExitStack() as ctx:
        const = ctx.enter_context(tc.tile_pool(name="const", bufs=1))
        psum = ctx.enter_context(tc.tile_pool(name="psum", bufs=2, space="PSUM"))
        o1p = ctx.enter_context(tc.tile_pool(name="oexp", bufs=3))
        op = ctx.enter_context(tc.tile_pool(name="omul", bufs=3))

        xt_sb = const.tile([P, DCH, NL], mybir.dt.float8e4, tag="xt")
        yt_sb = const.tile([P, DCH, ML], mybir.dt.float8e4, tag="yt")
        ey2_sb = const.tile([P, ML], mybir.dt.bfloat16, tag="ey2")
        bias_sb = const.tile([P, NCH], mybir.dt.float32, tag="bias")

        xt_r = xt_d.rearrange("(c p) n -> p c n", p=P)
        yt_r = yt_d.rearrange("(c p) n -> p c n", p=P)

        # Input DMAs in first-use order with a fine-grained head so the first
        # matmuls are gated on as few bytes as possible: iteration (mh=0,n=0)
        # needs xt d-chunks 0-1 cols 0:512 and yt d-chunks 0-1 cols 0:2048
        # (b=0), then d-chunks 2-3 (b=1); the second m-half's yt and ey2
        # stream in behind the first half's compute.
        nc.sync.dma_start(bias_sb[:], bias_d[:])
        nc.sync.dma_start(xt_sb[:, 0:2, :FREE], xt_r[:, 0:2, :FREE])
        nc.sync.dma_start(yt_sb[:, 0:2, :FREE], yt_r[:, 0:2, :FREE])
        nc.sync.dma_start(yt_sb[:, 0:2, FREE:HW], yt_r[:, 0:2, FREE:HW])
        nc.sync.dma_start(yt_sb[:, 2:4, :FREE], yt_r[:, 2:4, :FREE])
        nc.sync.dma_start(yt_sb[:, 2:4, FREE:HW], yt_r[:, 2:4, FREE:HW])
        nc.sync.dma_start(xt_sb[:, 2:4, :FREE], xt_r[:, 2:4, :FREE])
        nc.sync.dma_start(xt_sb[:, 0:2, FREE:], xt_r[:, 0:2, FREE:])
        nc.sync.dma_start(xt_sb[:, 2:4, FREE:], xt_r[:, 2:4, FREE:])
        nc.sync.dma_start(ey2_sb[:, :HW], ey2_d[:, :HW])
        nc.sync.dma_start(yt_sb[:, 0:2, HW:], yt_r[:, 0:2, HW:])
        nc.sync.dma_start(yt_sb[:, 2:4, HW:], yt_r[:, 2:4, HW:])
        nc.sync.dma_start(ey2_sb[:, HW:], ey2_d[:, HW:])

        # m-halves as the outer loop: steady state needs only half of yt
        # loaded; the other half streams in during the first half's compute.
        for mh in range(HB):
            for n in range(NCH):
                pt = psum.tile([P, HW], mybir.dt.float32, tag="pt",
                               name=f"pt_{mh}_{n}")
                o1 = o1p.tile([P, HW], mybir.dt.bfloat16, tag="o1",
                              name=f"o1_{mh}_{n}")
                for b in range(KB):
                    for mi in range(MH):
                        m = mh * MH + mi
                        nc.tensor.matmul(
                            pt[:, mi * FREE:(mi + 1) * FREE],
                            xt_sb[:, 2 * b:2 * b + 2, n * P:(n + 1) * P],
                            yt_sb[:, 2 * b:2 * b + 2, m * FREE:(m + 1) * FREE],
                            start=(b == 0),
                            stop=(b == KB - 1),
                            perf_mode=mybir.MatmulPerfMode.DoubleRow,
                        )
                o = op.tile([P, HW], mybir.dt.bfloat16, tag="o",
                            name=f"o_{mh}_{n}")
                if mh < HB - 1 or n < NCH - 1:
                    nc.scalar.activation(
                        o1[:], pt[:],
                        mybir.ActivationFunctionType.Exp,
                        bias=bias_sb[:, n:n + 1], scale=1.0,
                    )
                    nc.vector.tensor_mul(
                        o[:], o1[:], ey2_sb[:, mh * HW:(mh + 1) * HW]
                    )
                    nc.sync.dma_start(
                        out_d[n * P:(n + 1) * P, mh * HW:(mh + 1) * HW], o[:]
                    )
                else:
                    # very last tile: half-grained epilogue shortens the
                    # final serial ACT -> DVE -> DMA drain
                    QW = HW // 2
                    for q in range(2):
                        sl = slice(q * QW, (q + 1) * QW)
                        nc.scalar.activation(
                            o1[:, sl], pt[:, sl],
                            mybir.ActivationFunctionType.Exp,
                            bias=bias_sb[:, n:n + 1], scale=1.0,
                        )
                        nc.vector.tensor_mul(
                            o[:, sl], o1[:, sl],
                            ey2_sb[:, mh * HW + q * QW:mh * HW + (q + 1) * QW],
                        )
                        nc.sync.dma_start(
                            out_d[n * P:(n + 1) * P,
                                  mh * HW + q * QW:mh * HW + (q + 1) * QW],
                            o[:, sl],
                        )

    nc.compile()
    return nc


def kernel(x, y, gamma):
    global last_results
    x = np.asarray(x, dtype=np.float32).reshape(N, D)
    y = np.asarray(y, dtype=np.float32).reshape(M, D)
    g = float(np.asarray(gamma, dtype=np.float32).reshape(-1)[0])

    x2 = np.einsum("nd,nd->n", x, x, dtype=np.float32)
    y2 = np.einsum("md,md->m", y, y, dtype=np.float32)
    xt = np.ascontiguousarray((x * np.float32(2.0 * g)).T).astype(ml_dtypes.float8_e4m3)
    yt = np.ascontiguousarray(y.T).astype(ml_dtypes.float8_e4m3)
    bias = (-g * x2 - np.float32(SHIFT)).astype(np.float32)       # ACT bias
    ey2 = np.exp(SHIFT - g * y2.astype(np.float64))               # column factor
    ey2 = ey2.astype(ml_dtypes.bfloat16)

    in_maps = []
    for k in range(8):
        i, j = divmod(k, YS)
        in_maps.append({
            "xt": np.ascontiguousarray(xt[:, i * NL:(i + 1) * NL]),
            "yt": np.ascontiguousarray(yt[:, j * ML:(j + 1) * ML]),
            "ey2": np.ascontiguousarray(
                np.broadcast_to(ey2[j * ML:(j + 1) * ML], (P, ML))
            ),
            "biast": np.ascontiguousarray(
                bias[i * NL:(i + 1) * NL].reshape(NCH, P).T
            ),
        })

    if "nc" not in _CACHE:
        _CACHE["nc"] = _build_nc()
    nc = _CACHE["nc"]

    trace = os.environ.get("KERNEL_TRACE", "0") == "1"
    last_results = run_bass_kernel_spmd(nc, in_maps, list(range(8)), trace=trace)

    out = np.empty((N, M), dtype=np.float32)
    for k in range(8):
        i, j = divmod(k, YS)
        out[i * NL:(i + 1) * NL, j * ML:(j + 1) * ML] = (
            last_results.results[k]["out"].astype(np.float32)
        )
    return out


if __name__ == "__main__":
    t0 = time.time()
    rng = np.random.default_rng(0)
    x = rng.standard_normal((N, D), dtype=np.float32)
    y = rng.standard_normal((M, D), dtype=np.float32)
    gamma = np.ones((1,), dtype=np.float32)
    out = kernel(x, y, gamma)
    print(f"kernel() wall: {time.time()-t0:.1f}s; out[0,:4]={out[0, :4]}")


# revision 47
# speedup vs baseline: 1.1773x; 1.0117x over previous
"""RBF Gram-matrix kernel for Trainium2 (8 NeuronCores, SPMD).

Computes out[n, m] = exp(-gamma * ||x_n - y_m||^2) for x: [8192, 512],
y: [8192, 512] via the GEMM identity ||x-y||^2 = x2 + y2 - 2*x.y.

Sharding: 4x2 grid over the 8 cores — x rows split in 4 shards of 2048,
y rows split in 2 shards of 4096. Each core computes a [2048, 4096] tile
of the full [8192, 8192] output.

Device kernel per core, using the multiplicative split
  exp(-g||x-y||^2) = exp(2g x.y - g x2[n] - S) * e^{S - g y2[m]}:
  psum[n, m]  = sum_d (2g*x)^T[d, n] * y^T[d, m]        (TensorE, fp8e4
                DoubleRow: 256-deep contraction per matmul, f32 acc)
  o1          = exp(psum + (-g*x2[n] - S))               (ScalarE LUT,
                reads PSUM, per-partition bias, bf16 out)
  o           = o1 * e^{S - g*y2}[m]                     (VectorE TT mult,
                all-bf16 SBUF -> 2x perf mode)
The shift S keeps the exp argument comfortably below bf16 overflow so the
product can never be Inf*0.  Output travels as bf16 (halves the 32 MB/core
drain DMA) and is upcast to f32 on the host.

Schedule: m-halves outer, n-chunks inner, one [128, 2048] PSUM tile
(4 banks, double-buffered) per iteration; the 2048-wide ACT drain is the
saturated engine (~99% busy steady-state); TensorE runs DoubleRow matmuls
at ~216 ns each with LDWEIGHTS fully hidden; output leaves as one
[128, 2048] bf16 DMA per iteration (4KB DRAM rows -> full-rate packets).
"""
import os
import time
from contextlib import ExitStack

import numpy as np
import ml_dtypes

import concourse.mybir as mybir
import concourse.tile as tile
from concourse import bacc
from concourse.bass_utils import run_bass_kernel_spmd

N, M, D = 8192, 8192, 512
XS, YS = 4, 2              # shard grid: 4 x-shards x 2 y-shards = 8 cores
NL, ML = N // XS, M // YS  # per-core output tile: [2048, 4096]
P = 128
DCH = D // P               # 4 contraction subtiles of 128
KB = DCH // 2              # 2 DoubleRow chunks, 256-deep each
NCH = NL // P              # 16 row chunks of 128
FREE = 512                 # matmul free dim = one PSUM bank of f32
MT = ML // FREE            # 8 column tiles
HB = 2                     # m-halves: 2 psum tiles of 4 banks per n-chunk
MH = MT // HB              # 4 m-tiles per half
HW = MH * FREE             # 2048 cols per half
SHIFT = 50.0               # exponent rebalance between the two factors

_CACHE = {}
last_results = None        # BassKernelResults of the most recent run (for test.py)


def _build_nc():
    nc = bacc.Bacc("TRN2", target_bir_lowering=False, debug=False, num_devices=8)
    xt_d = nc.dram_tensor("xt", [D, NL], mybir.dt.float8e4, kind="ExternalInput").ap()
    yt_d = nc.dram_tensor("yt", [D, ML], mybir.dt.float8e4, kind="ExternalInput").ap()
    ey2_d = nc.dram_tensor("ey2", [P, ML], mybir.dt.bfloat16, kind="ExternalInput").ap()
    bias_d = nc.dram_tensor("biast", [P, NCH], mybir.dt.float32, kind="ExternalInput").ap()
    out_d = nc.dram_tensor("out", [NL, ML], mybir.dt.bfloat16, kind="ExternalOutput").ap()

    with tile.TileContext(nc) as tc, ExitStack() as ctx:
        const = ctx.enter_context(tc.tile_pool(name="const", bufs=1))
        psum = ctx.enter_context(tc.tile_pool(name="psum", bufs=2, space="PSUM"))
        o1p = ctx.enter_context(tc.tile_pool(name="oexp", bufs=3))
        op = ctx.enter_context(tc.tile_pool(name="omul", bufs=3))

        xt_sb = const.tile([P, DCH, NL], mybir.dt.float8e4, tag="xt")
        yt_sb = const.tile([P, DCH, ML], mybir.dt.float8e4, tag="yt")
        ey2_sb = const.tile([P, ML], mybir.dt.bfloat16, tag="ey2")
        bias_sb = const.tile([P, NCH], mybir.dt.float32, tag="bias")

        xt_r = xt_d.rearrange("(c p) n -> p c n", p=P)
        yt_r = yt_d.rearrange("(c p) n -> p c n", p=P)

        # Input DMAs in first-use order with a fine-grained head so the first
        # matmuls are gated on as few bytes as possible: iteration (mh=0,n=0)
        # needs xt d-chunks 0-1 cols 0:512 and yt d-chunks 0-1 cols 0:2048
        # (b=0), then d-chunks 2-3 (b=1); the second m-half's yt and ey2
        # stream in behind the first half's compute.
        nc.sync.dma_start(xt_sb[:, 0:2, :FREE], xt_r[:, 0:2, :FREE])
        nc.sync.dma_start(yt_sb[:, 0:2, :FREE], yt_r[:, 0:2, :FREE])
        nc.sync.dma_start(yt_sb[:, 0:2, FREE:HW], yt_r[:, 0:2, FREE:HW])
        nc.sync.dma_start(yt_sb[:, 2:4, :HW], yt_r[:, 2:4, :HW])
        nc.sync.dma_start(bias_sb[:], bias_d[:])
        nc.sync.dma_start(xt_sb[:, 2:4, :FREE], xt_r[:, 2:4, :FREE])
        nc.sync.dma_start(xt_sb[:, 0:2, FREE:], xt_r[:, 0:2, FREE:])
        nc.sync.dma_start(xt_sb[:, 2:4, FREE:], xt_r[:, 2:4, FREE:])
        nc.sync.dma_start(ey2_sb[:, :HW], ey2_d[:, :HW])
        nc.sync.dma_start(yt_sb[:, 0:2, HW:], yt_r[:, 0:2, HW:])
        nc.sync.dma_start(yt_sb[:, 2:4, HW:], yt_r[:, 2:4, HW:])
        nc.sync.dma_start(ey2_sb[:, HW:], ey2_d[:, HW:])

        # m-halves as the outer loop: steady state needs only half of yt
        # loaded; the other half streams in during the first half's compute.
        for mh in range(HB):
            for n in range(NCH):
                pt = psum.tile([P, HW], mybir.dt.float32, tag="pt",
                               name=f"pt_{mh}_{n}")
                o1 = o1p.tile([P, HW], mybir.dt.bfloat16, tag="o1",
                              name=f"o1_{mh}_{n}")
                for b in range(KB):
                    for mi in range(MH):
                        m = mh * MH + mi
                        nc.tensor.matmul(
                            pt[:, mi * FREE:(mi + 1) * FREE],
                            xt_sb[:, 2 * b:2 * b + 2, n * P:(n + 1) * P],
                            yt_sb[:, 2 * b:2 * b + 2, m * FREE:(m + 1) * FREE],
                            start=(b == 0),
                            stop=(b == KB - 1),
                            perf_mode=mybir.MatmulPerfMode.DoubleRow,
                        )
                o = op.tile([P, HW], mybir.dt.bfloat16, tag="o",
                            name=f"o_{mh}_{n}")
                if mh < HB - 1 or n < NCH - 1:
                    nc.scalar.activation(
                        o1[:], pt[:],
                        mybir.ActivationFunctionType.Exp,
                        bias=bias_sb[:, n:n + 1], scale=1.0,
                    )
                    nc.vector.tensor_mul(
                        o[:], o1[:], ey2_sb[:, mh * HW:(mh + 1) * HW]
                    )
                    nc.sync.dma_start(
                        out_d[n * P:(n + 1) * P, mh * HW:(mh + 1) * HW], o[:]
                    )
                else:
                    # very last tile: half-grained epilogue shortens the
                    # final serial ACT -> DVE -> DMA drain
                    QW = HW // 2
                    for q in range(2):
                        sl = slice(q * QW, (q + 1) * QW)
                        nc.scalar.activation(
                            o1[:, sl], pt[:, sl],
                            mybir.ActivationFunctionType.Exp,
                            bias=bias_sb[:, n:n + 1], scale=1.0,
                        )
                        nc.vector.tensor_mul(
                            o[:, sl], o1[:, sl],
                            ey2_sb[:, mh * HW + q * QW:mh * HW + (q + 1) * QW],
                        )
                        nc.sync.dma_start(
                            out_d[n * P:(n + 1) * P,
                                  mh * HW + q * QW:mh * HW + (q + 1) * QW],
                            o[:, sl],
                        )

    nc.compile()
    return nc


def kernel(x, y, gamma):
    global last_results
    x = np.asarray(x, dtype=np.float32).reshape(N, D)
    y = np.asarray(y, dtype=np.float32).reshape(M, D)
    g = float(np.asarray(gamma, dtype=np.float32).reshape(-1)[0])

    x2 = np.einsum("nd,nd->n", x, x, dtype=np.float32)
    y2 = np.einsum("md,md->m", y, y, dtype=np.float32)
    xt = np.ascontiguousarray((x * np.float32(2.0 * g)).T).astype(ml_dtypes.float8_e4m3)
    yt = np.ascontiguousarray(y.T).astype(ml_dtypes.float8_e4m3)
    bias = (-g * x2 - np.float32(SHIFT)).astype(np.float32)       # ACT bias
    ey2 = np.exp(SHIFT - g * y2.astype(np.float64))               # column factor
    ey2 = ey2.astype(ml_dtypes.bfloat16)

    in_maps = []
    for k in range(8):
        i, j = divmod(k, YS)
        in_maps.append({
            "xt": np.ascontiguousarray(xt[:, i * NL:(i + 1) * NL]),
            "yt": np.ascontiguousarray(yt[:, j * ML:(j + 1) * ML]),
            "ey2": np.ascontiguousarray(
                np.broadcast_to(ey2[j * ML:(j + 1) * ML], (P, ML))
            ),
            "biast": np.ascontiguousarray(
                bias[i * NL:(i + 1) * NL].reshape(NCH, P).T
            ),
        })

    if "nc" not in _CACHE:
        _CACHE["nc"] = _build_nc()
    nc = _CACHE["nc"]

    trace = os.environ.get("KERNEL_TRACE", "0") == "1"
    last_results = run_bass_kernel_spmd(nc, in_maps, list(range(8)), trace=trace)

    out = np.empty((N, M), dtype=np.float32)
    for k in range(8):
        i, j = divmod(k, YS)
        out[i * NL:(i + 1) * NL, j * ML:(j + 1) * ML] = (
            last_results.results[k]["out"].astype(np.float32)
        )
    return out


if __name__ == "__main__":
    t0 = time.time()
    rng = np.random.default_rng(0)
    x = rng.standard_normal((N, D), dtype=np.float32)
    y = rng.standard_normal((M, D), dtype=np.float32)
    gamma = np.ones((1,), dtype=np.float32)
    out = kernel(x, y, gamma)
    print(f"kernel() wall: {time.time()-t0:.1f}s; out[0,:4]={out[0, :4]}")
